# revision 1
# baseline (speedup 1.0000x reference)
"""Cosine-similarity multi-head attention on 8 Trainium2 NeuronCores.

Sharding: tensor-parallel over (batch, head-group). Core c (c = b*4 + hg)
computes heads [4*hg, 4*hg+4) of batch b for ALL 2048 query tokens, then a
partial output projection over its 256 inner features.  The host sums the 4
partial outputs per batch and adds b_out (the "all-reduce" of the hint, done
during the host-side gather).  No K/V duplication: each projection row is
computed exactly once across the machine.

Per-core layouts:
  - xt   [128, 8, 2048]  x[b]^T, feature-chunked (bf16)
  - qnT/knT [128, 2, 2048] Q^T/K^T: chunk m holds local heads 2m (parts 0:64)
    and 2m+1 (parts 64:128); normalized in place (f32r)
  - av   [128, 16, 4, 65] V token-major per (key-chunk, head) + ones column
    so each A@V matmul also accumulates the softmax denominators
  - softmax: no max-subtraction (|logits| <= 10, exp safe in f32)
  - norm factors are broadcast across partitions with tiny K=1 PE matmuls
    (outer product with a ones row) -- no DRAM round trips.
"""

import numpy as np

B, N, DIM, H, DH = 2, 2048, 1024, 16, 64
INNER = H * DH
P = 128
KC = DIM // P        # 8 contraction chunks of the model dim
JC = N // P          # 16 key-token chunks of 128
QB = 4               # query blocks of 512
NQ = N // QB         # 512
HL = 4               # heads per core
M = 2                # feature chunks per core (4 heads * 64 = 256)
MAX_LOG_SCALE = float(np.log(1.0 / 0.01))

_CACHE = {}


def _build():
    if "nc" in _CACHE:
        return _CACHE["nc"]
    import concourse.bass as bass
    import concourse.bacc as bacc
    import concourse.mybir as mybir
    import concourse.tile as tile

    f32 = mybir.dt.float32
    f32r = mybir.dt.float32r
    bf16 = mybir.dt.bfloat16
    AF = mybir.ActivationFunctionType

    nc = bacc.Bacc("TRN2", target_bir_lowering=False)

    xTb = nc.declare_dram_parameter("xTb", [P, KC, N], bf16, isOutput=False)
    wqb = nc.declare_dram_parameter("wqb", [P, KC, M, P], bf16, isOutput=False)
    wkb = nc.declare_dram_parameter("wkb", [P, KC, M, P], bf16, isOutput=False)
    wvb = nc.declare_dram_parameter("wvb", [P, KC, M * P], bf16, isOutput=False)
    wob = nc.declare_dram_parameter("wob", [P, M, KC, P], bf16, isOutput=False)
    hsq = nc.declare_dram_parameter("hsq", [P, M, P], bf16, isOutput=False)
    sclq2 = nc.declare_dram_parameter("sclq2", [P, M], f32, isOutput=False)
    oned = nc.declare_dram_parameter("oned", [P, JC, HL, 64], f32r,
                                     isOutput=False)
    outT = nc.declare_dram_parameter("outT", [DIM, N], f32, isOutput=True)

    with tile.TileContext(nc) as tc:
        with (
            tc.tile_pool(name="persist", bufs=1) as pp,
            tc.tile_pool(name="work", bufs=2) as pa,
            tc.tile_pool(name="pout", bufs=4) as pout,
            tc.tile_pool(name="ps", bufs=2, space="PSUM") as ps,
        ):
            xt = pp.tile([P, KC, N], bf16, tag="xt")
            qnT = pp.tile([P, M, N], bf16, tag="qnT")
            knT = pp.tile([P, M, N], bf16, tag="knT")
            av = pp.tile([P, JC, HL, P], f32r, tag="av")
            onT = pp.tile([P, M, N], bf16, tag="onT")
            wq_sb = pp.tile([P, KC, M, P], bf16, tag="wq")
            wk_sb = pp.tile([P, KC, M, P], bf16, tag="wk")
            wv_sb = pp.tile([P, KC, M * P], bf16, tag="wv")
            wo_sb = pp.tile([P, M, KC, P], bf16, tag="wo")
            hsq_sb = pp.tile([P, M, P], bf16, tag="hsq")
            scl_sb = pp.tile([P, M], f32, tag="scl")
            zero_b = pp.tile([P, 1], f32, tag="zerob")

            nc.sync.dma_start(out=wq_sb[:], in_=wqb[:])
            nc.sync.dma_start(out=hsq_sb[:], in_=hsq[:])
            nc.sync.dma_start(out=scl_sb[:], in_=sclq2[:])
            nc.vector.memset(zero_b[:], 0.0)
            nc.gpsimd.dma_start(out=av[:, :, :, 0:DH], in_=oned[:])
            for tq in range(QB):
                eng = nc.sync if tq % 2 == 0 else nc.gpsimd
                eng.dma_start(out=xt[:, :, tq * NQ : (tq + 1) * NQ],
                              in_=xTb[:, :, tq * NQ : (tq + 1) * NQ])
            nc.sync.dma_start(out=wk_sb[:], in_=wkb[:])
            nc.gpsimd.dma_start(out=wv_sb[:], in_=wvb[:])
            nc.gpsimd.dma_start(out=wo_sb[:], in_=wob[:])

            # ---------------- Phase A: Q/K projections + norms ------------
            def factor_apply(nT, sqs_list, qb):
                # sqs rows hold |q| (or |k|) replicated across each head's 64
                # partitions; approx-reciprocal then scale in place
                qs = slice(qb * NQ, (qb + 1) * NQ)
                for m in range(M):
                    sqf = pa.tile([P, NQ], f32, tag="sqf")
                    nc.vector.reciprocal_approx_fast(
                        out=sqf[:], in_=sqs_list[m][:])
                    nc.vector.tensor_mul(nT[:, m, qs], sqf[:], nT[:, m, qs])

            def emit_norm(pend_sq, q_scale):
                psq, pm, plist = pend_sq
                pn = ps.tile([P, NQ], f32, tag="avp", name="pn")
                nc.tensor.matmul(pn[:], hsq_sb[:, pm, :], psq[:],
                                 start=True, stop=True)
                sqs = pa.tile([P, NQ], f32, tag="sqs", bufs=4)
                nc.scalar.activation(
                    sqs[:], pn[:], AF.Sqrt, bias=zero_b[:],
                    scale=scl_sb[:, pm : pm + 1] if q_scale else 1.0)
                plist.append(sqs)

            def proj_side(w_sb, nT, q_scale, evac_eng):
                # norm matmul for each (qb, m) emitted one step late so the
                # PE never waits in-order on the DVE/Act square chain
                pend = None
                pend_sq = None
                for qb in range(QB):
                    qs = slice(qb * NQ, (qb + 1) * NQ)
                    sqs_list = []
                    for m in range(M):
                        pq = ps.tile([P, NQ], f32, tag="pq", name="pq")
                        for kc in range(KC):
                            nc.tensor.matmul(pq[:], w_sb[:, kc, m, :],
                                             xt[:, kc, qs],
                                             start=(kc == 0),
                                             stop=(kc == KC - 1))
                        if evac_eng == 0:
                            nc.vector.tensor_copy(nT[:, m, qs], pq[:])
                        else:
                            nc.scalar.copy(nT[:, m, qs], pq[:])
                        sq = pa.tile([P, NQ], bf16, tag="sq")
                        nc.vector.tensor_mul(sq[:], pq[:], nT[:, m, qs])
                        if pend_sq is not None:
                            emit_norm(pend_sq, q_scale)
                        pend_sq = (sq, m, sqs_list)
                    if pend is not None:
                        factor_apply(nT, *pend)
                    pend = (sqs_list, qb)
                emit_norm(pend_sq, q_scale)
                factor_apply(nT, *pend)

            proj_side(wq_sb, qnT, True, 0)   # Q: temp folded into sqrt scale
            proj_side(wk_sb, knT, False, 1)  # K: evacuate via Act engine

            # ------------- Phase B: attention + V-weave + out-proj --------
            def emit_v(jc):
                pv = ps.tile([P, M * P], f32, tag="pq", name="pv")
                for kc in range(KC):
                    nc.tensor.matmul(pv[:], xt[:, kc, jc * P : (jc + 1) * P],
                                     wv_sb[:, kc, :],
                                     start=(kc == 0), stop=(kc == KC - 1))
                nc.vector.tensor_copy(
                    av[:, jc, :, DH:P],
                    pv[:].rearrange("p (h d) -> p h d", d=DH))

            def emit_outproj(qb, mo):
                qs = slice(qb * NQ, (qb + 1) * NQ)
                cp = ps.tile([P, NQ], f32, tag="pq", name=f"cp{mo}")
                for g in range(M):
                    nc.tensor.matmul(cp[:], wo_sb[:, g, mo, :], onT[:, g, qs],
                                     start=(g == 0), stop=(g == M - 1))
                oc = pout.tile([P, NQ], f32, tag="ot", name=f"oc{mo}")
                nc.vector.tensor_copy(oc[:], cp[:])
                nc.sync.dma_start(out=outT[mo * P : (mo + 1) * P, qs],
                                  in_=oc[:])

            for qb in range(QB):
                qs = slice(qb * NQ, (qb + 1) * NQ)
                for pr in range(M):       # head pair (2pr, 2pr+1)
                    avp0 = ps.tile([P, NQ], f32, tag="avp", name="avp0")
                    avp1 = ps.tile([P, NQ], f32, tag="avp", name="avp1")
                    pend_et = None
                    for kc in range(JC):
                        ks = slice(kc * P, (kc + 1) * P)
                        if qb == 0 and pr == 0:
                            emit_v(kc)    # weave V projection into sweep 0
                        sp = ps.tile([P, 2, NQ], f32, tag="sps")
                        nc.tensor.matmul(sp[:, 0, :], knT[0:64, pr, ks],
                                         qnT[0:64, pr, qs],
                                         start=True, stop=True)
                        nc.tensor.matmul(sp[:, 1, :], knT[64:P, pr, ks],
                                         qnT[64:P, pr, qs],
                                         start=True, stop=True)
                        # A@V runs one kc behind so the PE never waits
                        # in-order on the exp it just requested
                        if pend_et is not None:
                            pet, pkc = pend_et
                            nc.tensor.matmul(avp0[:], av[:, pkc, 2 * pr, :],
                                             pet[:, 0, :],
                                             start=(pkc == 0), stop=False)
                            nc.tensor.matmul(avp1[:], av[:, pkc, 2 * pr + 1, :],
                                             pet[:, 1, :],
                                             start=(pkc == 0), stop=False)
                        et = pa.tile([P, 2, NQ], f32r, tag="et", bufs=4)
                        nc.scalar.activation(et[:], sp[:], AF.Exp,
                                             bias=zero_b[:])
                        pend_et = (et, kc)
                        # spread the previous block's out-projection matmuls
                        # across BOTH sweeps to even the PE load
                        if qb > 0 and kc % 4 == 3:
                            emit_outproj(qb - 1, 4 * pr + kc // 4)
                    pet, pkc = pend_et
                    nc.tensor.matmul(avp0[:], av[:, pkc, 2 * pr, :],
                                     pet[:, 0, :], start=False, stop=True)
                    nc.tensor.matmul(avp1[:], av[:, pkc, 2 * pr + 1, :],
                                     pet[:, 1, :], start=False, stop=True)
                    # denominators arrive replicated on partitions 0:64 (ones
                    # block in av); approx-recip + normalize in place
                    rec0 = pa.tile([64, NQ], f32, tag="dn", name="rec0")
                    nc.vector.reciprocal_approx_fast(
                        out=rec0[:], in_=avp0[0:DH, :])
                    nc.vector.tensor_mul(onT[0:64, pr, qs], avp0[DH:P, :],
                                         rec0[:])
                    rec1 = pa.tile([64, NQ], f32, tag="dn", name="rec1")
                    nc.vector.reciprocal_approx_fast(
                        out=rec1[:], in_=avp1[0:DH, :])
                    nc.vector.tensor_mul(onT[64:P, pr, qs], avp1[DH:P, :],
                                         rec1[:])
            for mo in range(KC):
                emit_outproj(QB - 1, mo)

    nc.compile()
    _CACHE["nc"] = nc
    return nc


def run(inputs, trace=False):
    import ml_dtypes
    from concourse.bass_utils import run_bass_kernel_spmd

    x = np.asarray(inputs["x"], np.float32)
    w_qkv = np.asarray(inputs["w_qkv"], np.float32)
    w_out = np.asarray(inputs["w_out"], np.float32)
    b_out = np.asarray(inputs["b_out"], np.float32)
    logit_scale = np.asarray(inputs["logit_scale"], np.float32)

    nc = _build()
    bf = ml_dtypes.bfloat16

    scl = np.exp(np.minimum(logit_scale.reshape(H), MAX_LOG_SCALE))

    xTb = [np.ascontiguousarray(
        x[b].T.reshape(KC, P, N).transpose(1, 0, 2)).astype(bf)
        for b in range(B)]

    in_maps = []
    for c in range(8):
        b, hg = c // 4, c % 4
        cs = slice(hg * 256, (hg + 1) * 256)
        wq = np.ascontiguousarray(
            w_qkv[:, 0:INNER][:, cs].reshape(KC, P, M, P)
            .transpose(1, 0, 2, 3)).astype(bf)
        wk = np.ascontiguousarray(
            w_qkv[:, INNER:2 * INNER][:, cs].reshape(KC, P, M, P)
            .transpose(1, 0, 2, 3)).astype(bf)
        wv = np.ascontiguousarray(
            w_qkv[:, 2 * INNER:3 * INNER][:, cs].reshape(KC, P, M * P)
            .transpose(1, 0, 2)).astype(bf)
        wo = np.ascontiguousarray(
            w_out[cs, :].reshape(M, P, KC, P).transpose(1, 0, 2, 3)).astype(bf)
        hs = np.zeros((P, M, P), bf)
        hs[0:64, :, 0:64] = 1.0
        hs[64:P, :, 64:P] = 1.0
        sc2 = np.empty((P, M), np.float32)
        for m in range(M):
            sc2[0:64, m] = scl[4 * hg + 2 * m] ** -2.0
            sc2[64:P, m] = scl[4 * hg + 2 * m + 1] ** -2.0
        in_maps.append({
            "xTb": xTb[b], "wqb": wq, "wkb": wk, "wvb": wv, "wob": wo,
            "hsq": hs, "sclq2": sc2,
            "oned": np.ones((P, JC, HL, 64), np.float32),
        })

    res = run_bass_kernel_spmd(nc, in_maps, list(range(8)), trace=trace)

    out = np.empty((B, N, DIM), np.float32)
    for b in range(B):
        acc = res.results[4 * b]["outT"].astype(np.float32)
        for hg in range(1, 4):
            acc = acc + res.results[4 * b + hg]["outT"]
        out[b] = acc.T + b_out
    return out, res


def kernel(**inputs):
    out, _ = run(inputs, trace=False)
    return out

